# revision 1
# baseline (speedup 1.0000x reference)
"""Binarized 3x3 conv (BConv2d) on 8 TRN2 NeuronCores.

Problem: x (32, 32, 256, 256) f32, weight (32, 32, 3, 3) f32.
  out = conv2d(x, sign(weight), padding='same') / sqrt(32*9)

Strategy:
  - Data-parallel over batch: core i gets images 4i..4i+3 (no collectives).
  - Per core, pack 4 images x 32 input channels onto the 128 SBUF
    partitions.  Each 3x3 tap becomes ONE K=128, M=128 matmul with a
    block-diagonal (per-image) binarized weight matrix; the 9 taps
    accumulate into PSUM and differ only in the rhs address offset into a
    zero-padded fp16 copy of the input (258x258 per partition).
  - fp16 inputs (weights are exactly +-1 in fp16), fp32 PSUM accumulate,
    fp16 output (host upcasts to f32; ~3e-4 rel err vs 2e-2 gate).
  - x is cast to fp16 host-side so DMA writes straight into the padded
    image (no staging tile, no cast pass, half the input HBM traffic).
  - Pipeline: DMA fp16 rows -> padded image, 9x4 matmuls per 8-row
    super-chunk into 4 PSUM banks (tapered 4,2,2 at the end), VectorE
    scaled drain to fp16, DMA out.  Zero-weight warm-up matmuls keep the
    PE clock gate hot while the first input rows are in flight.
"""

import numpy as np
import ml_dtypes

import concourse.mybir as mybir
import concourse.tile as tile
from concourse import bacc
from concourse import bass_utils

N_CORES = 8
N_IMG = 4          # images per core
C_IN = 32
C_OUT = 32
K = 3
H = 256
W = 256
DIV = float(np.sqrt(C_IN * K * K))


def build_conv_kernel(
    nimg=N_IMG,
    cin=C_IN,
    cout=C_OUT,
    h=H,
    w=W,
    chunk_rows=16,  # input DMA granularity (rows; fp16 -> 1 MB per DMA)
    bank_rows=2,    # output rows per PSUM bank matmul (bank_rows*w <= 512)
    banks_per_sc=4, # PSUM banks per super-chunk
    div=DIV,
    repeats=1,      # execute the whole body N times (for delta-timing)
    warmup_mms=8,   # zero-weight matmuls to warm the PE during input wait
):
    """Build the per-core Bass graph.  Returns nc (compiled Bacc)."""
    P = nimg * cin
    assert P <= 128
    M = nimg * cout
    assert M <= 128
    assert bank_rows * w <= 512
    sc_rows = bank_rows * banks_per_sc
    assert h % chunk_rows == 0 and h % sc_rows == 0
    hp, wp = h + 2, w + 2
    n_taps = 9

    nc = bacc.Bacc(
        "TRN2", target_bir_lowering=False, debug=False, num_devices=N_CORES
    )
    # x arrives pre-converted to fp16 (host-side marshalling): halves input
    # HBM traffic and lets DMA write straight into the padded image with no
    # staging tile or ScalarE cast pass.
    x_dram = nc.dram_tensor("x", [P, h, w], mybir.dt.float16, kind="ExternalInput")
    w_dram = nc.dram_tensor(
        "w9", [P, n_taps, M], mybir.dt.float16, kind="ExternalInput"
    )
    out_dram = nc.dram_tensor(
        "out", [M, h, w], mybir.dt.float16, kind="ExternalOutput"
    )

    with tile.TileContext(nc) as tc:
        with (
            tc.tile_pool(name="persist", bufs=1) as perpool,
            tc.tile_pool(name="ostage", bufs=3) as opool,
            tc.tile_pool(name="psum", bufs=2 * banks_per_sc, space="PSUM") as ppool,
        ):
            xpad = perpool.tile([P, hp, wp], mybir.dt.float16, name="xpad")
            wsb = perpool.tile([P, n_taps, M], mybir.dt.float16, name="wsb")
            wz = perpool.tile([P, M], mybir.dt.float16, name="wz")
            nc.vector.memset(wz[:], 0.0)

            # tap-0 weights first so the first LDWEIGHTS unblocks early
            nc.sync.dma_start(out=wsb[:, 0, :], in_=w_dram[:, 0, :])
            nc.sync.dma_start(out=wsb[:, 1:, :], in_=w_dram[:, 1:, :])
            # zero the padding ring (top/bottom rows, left/right columns)
            nc.vector.memset(xpad[:, 0, :], 0.0)
            nc.vector.memset(xpad[:, hp - 1, :], 0.0)
            nc.vector.memset(xpad[:, :, 0], 0.0)
            nc.vector.memset(xpad[:, :, wp - 1], 0.0)

            def emit_input_rows(r0, nrows):
                nc.sync.dma_start(
                    out=xpad[:, r0 + 1 : r0 + nrows + 1, 1 : w + 1],
                    in_=x_dram[:, r0 : r0 + nrows, :],
                )

            def emit_body():
                # Interleave input chunks with compute super-chunks so DMA
                # lane semaphore windows complete progressively (an
                # up-front input burst couples early sem resets to the
                # last input DMA and stalls the whole pipeline mid-kernel).
                next_row = [0]

                def load_until(row_needed):
                    while next_row[0] < min(row_needed, h):
                        r0 = next_row[0]
                        # small first pieces so the first matmuls start early
                        nrows = 2 if r0 < sc_rows else chunk_rows
                        nrows = min(nrows, h - r0)
                        emit_input_rows(r0, nrows)
                        next_row[0] += nrows

                # compute pipeline: super-chunks of output rows, one
                # PSUM-bank tile per bank_rows strip (own accum group).
                # The last super-chunk tapers (4,2,2 rows) so the final
                # drain->store chain after the last matmul is short.
                plan = []
                r = 0
                while r < h:
                    if h - r > sc_rows:
                        rows = sc_rows
                    elif h - r == sc_rows and sc_rows >= 8:
                        plan += [(r, sc_rows // 2), (r + sc_rows // 2,
                                                     sc_rows // 4)]
                        r += 3 * sc_rows // 4
                        rows = h - r
                    else:
                        rows = h - r
                    plan.append((r, rows))
                    r += rows

                # PE warm-up while the first input chunks are in flight:
                # zero-weight matmuls on the (memset) pad row keep the PE
                # busy so the HAM clock gate reaches 2.4 GHz before real
                # work.  Reads/writes only zeros; scratch bank is unused.
                if warmup_mms:
                    wpt = ppool.tile(
                        [M, bank_rows, w], mybir.dt.float32,
                        name="wpt", tag="pt",
                    )
                    for _ in range(warmup_mms):
                        nc.tensor.matmul(
                            wpt[:, 0, 0:w], wz[:], xpad[:, 0, 0:w],
                            start=True, stop=True,
                        )

                for si, (h0, rows) in enumerate(plan):
                    load_until(min(h0 + rows + sc_rows + 1, h))
                    banks = rows // bank_rows
                    pts = [
                        ppool.tile(
                            [M, bank_rows, w], mybir.dt.float32,
                            name="pt", tag="pt",
                        )
                        for _ in range(banks)
                    ]
                    # first SC: bank-outer so bank 0's accumulation (which
                    # needs only the first 3 input rows) completes first
                    if si == 0:
                        order = [(t, b) for b in range(banks)
                                 for t in range(n_taps)]
                    else:
                        order = [(t, b) for t in range(n_taps)
                                 for b in range(banks)]
                    for t, b in order:
                        dy, dx = t // 3, t % 3
                        hb = h0 + b * bank_rows
                        nc.tensor.matmul(
                            pts[b][:],
                            wsb[:, t, :],
                            xpad[:, hb + dy : hb + dy + bank_rows, dx : dx + w],
                            start=(t == 0),
                            stop=(t == n_taps - 1),
                        )
                    ot = opool.tile(
                        [M, rows, w], mybir.dt.float16, name="ot", tag="ot",
                        padded_shape=[M, sc_rows, w],
                    )
                    for b in range(banks):
                        nc.vector.tensor_scalar_mul(
                            ot[:, b * bank_rows : (b + 1) * bank_rows, :],
                            pts[b][:],
                            1.0 / div,
                        )
                    nc.sync.dma_start(
                        out=out_dram[:, h0 : h0 + rows, :], in_=ot[:]
                    )
                load_until(h)

            for _rep in range(repeats):
                emit_body()

    nc.compile()
    return nc


def make_weight_tensor(weight, nimg=N_IMG, cin=C_IN, cout=C_OUT):
    """Binarize + block-diagonalize: [cout,cin,3,3] f32 -> [nimg*cin, 9, nimg*cout] bf16."""
    n_taps = weight.shape[2] * weight.shape[3]
    wbin = np.where(weight > 0, 1.0, -1.0).astype(np.float32)
    # [co, ci, kh, kw] -> [ci, t, co]
    wt = wbin.reshape(cout, cin, n_taps).transpose(1, 2, 0)
    w9 = np.zeros((nimg * cin, n_taps, nimg * cout), dtype=np.float16)
    for i in range(nimg):
        w9[i * cin : (i + 1) * cin, :, i * cout : (i + 1) * cout] = wt
    return w9


def kernel(x, weight, trace=False, repeats=1, _nc_cache={}):
    """Full-input entry point: x (32,32,256,256) f32, weight (32,32,3,3) f32."""
    x = np.asarray(x, dtype=np.float32)
    # host-side fp16 marshalling (fp16 >= bf16 precision at identical speed)
    x = np.ascontiguousarray(x.astype(np.float16))
    weight = np.asarray(weight, dtype=np.float32)
    n_batch = x.shape[0]
    per_core = n_batch // N_CORES

    if repeats not in _nc_cache:
        _nc_cache[repeats] = build_conv_kernel(repeats=repeats)
    nc = _nc_cache[repeats]

    w9 = make_weight_tensor(weight)
    P = N_IMG * C_IN
    in_maps = [
        {
            "x": x[i * per_core : (i + 1) * per_core].reshape(P, H, W),
            "w9": w9,
        }
        for i in range(N_CORES)
    ]
    try:
        res = bass_utils.run_bass_kernel_spmd(
            nc, in_maps, core_ids=list(range(N_CORES)), trace=trace
        )
    except ModuleNotFoundError:
        # axon NTFF profiling hook unavailable in this environment
        res = bass_utils.run_bass_kernel_spmd(
            nc, in_maps, core_ids=list(range(N_CORES)), trace=False
        )
    out = np.concatenate(
        [r["out"].astype(np.float32).reshape(per_core, C_OUT, H, W)
         for r in res.results],
        axis=0,
    )
    if trace:
        kernel.last_results = res
    return out



# revision 12
# speedup vs baseline: 1.4362x; 1.4362x over previous
"""Binarized 3x3 conv (BConv2d) on 8 TRN2 NeuronCores — Winograd F(2,3) along H.

Problem: x (32, 32, 256, 256) f32, weight (32, 32, 3, 3) f32.
  out = conv2d(x, sign(weight), padding='same') / sqrt(32*9)

Strategy:
  - Data-parallel over batch: core i gets images 4i..4i+3 (no collectives).
  - Per core, pack 4 images x 32 input channels onto the 128 SBUF
    partitions; weights are block-diagonal (per-image) as in the direct
    kernel.
  - 1D Winograd F(2,3) over the kh (row) axis: for each output row pair
    (2r, 2r+1), four transformed input rows V0..V3 (cheap row add/subs on
    DVE/Pool) feed 4 PSUM accumulation chains m0..m3, each a 3-tap kw
    convolution (matmuls over K=128).  out[2r] = m0+m1+m2,
    out[2r+1] = m1-m2-m3 (signs/scale folded into the transformed
    weights).  This cuts PE column count 1.5x vs the 9-tap direct form:
    12 matmuls of N=512 per 2 row-pairs instead of 9 of N=512 per pair.
  - Engine split so nothing passes the PE: DVE computes V0/V1 + the
    output combine adds; Pool (GpSimd) computes V2/V3; Act drains
    m0,m1,m2 from PSUM to fp16; DVE's last add reads m3 straight from
    PSUM.
  - fp16 inputs/outputs (host-side fp16 marshalling), fp32 PSUM
    accumulate; transformed weights carry the 1/sqrt(288) scale and the
    /2 Winograd factors.
"""

import numpy as np
import ml_dtypes

import concourse.mybir as mybir
import concourse.tile as tile
from concourse import bacc
from concourse import bass_utils

N_CORES = 8
N_IMG = 4          # images per core
C_IN = 32
C_OUT = 32
K = 3
H = 256
W = 256
DIV = float(np.sqrt(C_IN * K * K))
N_WT = 12          # 4 winograd positions x 3 kw taps


def build_conv_kernel(
    nimg=N_IMG,
    cin=C_IN,
    cout=C_OUT,
    h=H,
    w=W,
    chunk_rows=16,  # input DMA granularity (rows)
    vblock=4,       # r-pairs per V-transform block (8 input rows)
    div=DIV,
    repeats=1,      # execute the whole body N times (for delta-timing)
    warmup_mms=40,  # zero-weight matmuls (N=256) to cover the input wait
):
    """Build the per-core Bass graph.  Returns nc (compiled Bacc)."""
    P = nimg * cin
    assert P <= 128
    M = nimg * cout
    assert M <= 128
    hp, wp = h + 2, w + 2
    nr = h // 2                     # number of output row pairs
    assert nr % vblock == 0

    nc = bacc.Bacc(
        "TRN2", target_bir_lowering=False, debug=False, num_devices=N_CORES
    )
    x_dram = nc.dram_tensor("x", [P, h, w], mybir.dt.float16, kind="ExternalInput")
    w_dram = nc.dram_tensor(
        "w12", [P, N_WT, M], mybir.dt.float16, kind="ExternalInput"
    )
    out_dram = nc.dram_tensor(
        "out", [M, h, w], mybir.dt.float16, kind="ExternalOutput"
    )

    with tile.TileContext(nc) as tc:
        with (
            tc.tile_pool(name="persist", bufs=1) as perpool,
            tc.tile_pool(name="vpool", bufs=2) as vpool,
            tc.tile_pool(name="stage", bufs=2) as spool,
            tc.tile_pool(name="ostage", bufs=3) as opool,
            tc.tile_pool(name="psum", bufs=8, space="PSUM") as ppool,
        ):
            xpad = perpool.tile([P, hp, wp], mybir.dt.float16, name="xpad")
            wsb = perpool.tile([P, N_WT, M], mybir.dt.float16, name="wsb")
            wz = perpool.tile([P, M], mybir.dt.float16, name="wz")
            wmr = perpool.tile([P, w], mybir.dt.float16, name="wmr")
            # warmup operands on DVE so the PE can start immediately
            nc.vector.memset(wz[:], 0.0)
            nc.vector.memset(wmr[:], 0.0)

            def emit_input_rows(r0, nrows):
                nc.sync.dma_start(
                    out=xpad[:, r0 + 1 : r0 + nrows + 1, 1 : w + 1],
                    in_=x_dram[:, r0 : r0 + nrows, :],
                )

            # first input piece before the (less urgent) weight DMAs
            emit_input_rows(0, 2 * vblock + 2)
            nc.sync.dma_start(out=wsb[:, 0:3, :], in_=w_dram[:, 0:3, :])
            nc.sync.dma_start(out=wsb[:, 3:, :], in_=w_dram[:, 3:, :])
            # zero the padding ring (top/bottom rows, left/right columns)
            nc.gpsimd.memset(xpad[:, 0, :], 0.0)
            nc.gpsimd.memset(xpad[:, hp - 1, :], 0.0)
            nc.gpsimd.memset(xpad[:, :, 0], 0.0)
            nc.gpsimd.memset(xpad[:, :, wp - 1], 0.0)

            def emit_body(first):
                next_row = [2 * vblock + 2 if first else 0]

                def load_until(row_needed):
                    while next_row[0] < min(row_needed, h):
                        r0 = next_row[0]
                        nrows = min(chunk_rows, h - r0)
                        emit_input_rows(r0, nrows)
                        next_row[0] += nrows

                # PE warm-up while the first input chunks are in flight.
                if warmup_mms:
                    wpt = ppool.tile(
                        [M, 2, w], mybir.dt.float32, name="wpt", tag="pt",
                    )
                    for _ in range(warmup_mms):
                        nc.tensor.matmul(
                            wpt[:, 0, 0:w], wz[:], wmr[:],
                            start=True, stop=True,
                        )

                def emit_vblock(b, sub=None):
                    """V transform for r-pairs [vblock*b, vblock*(b+1)).

                    V0 = xpad[2r]   - xpad[2r+2]          (DVE)
                    V1 = xpad[2r+1] + xpad[2r+2]          (DVE)
                    V2 = xpad[2r+2] - xpad[2r+1]          (Pool)
                    V3 = xpad[2r+1] - xpad[2r+3]          (Pool)
                    """
                    vt = vpool.tile(
                        [P, vblock, 4, wp], mybir.dt.float16, name="v", tag="v",
                    )
                    s = 2 * vblock * b  # xpad row of first d0
                    n = vblock

                    def rows(off):
                        stop = min(s + off + 2 * n, hp)
                        return xpad[:, s + off : stop : 2, :]

                    # engine assignment: first block all-DVE for fast start
                    eng2 = nc.vector if b == 0 else nc.gpsimd
                    nc.vector.tensor_sub(vt[:, :, 0, :], rows(0), rows(2))
                    nc.vector.tensor_add(vt[:, :, 1, :], rows(1), rows(2))
                    eng2.tensor_sub(vt[:, :, 2, :], rows(2), rows(1))
                    eng2.tensor_sub(vt[:, :, 3, :], rows(1), rows(3))
                    return vt

                # groups of 2 r-pairs; final group split in two 1-pair
                # groups so the post-matmul drain tail is short
                plan = [(r0, 2) for r0 in range(0, nr - 2, 2)]
                plan += [(nr - 2, 1), (nr - 1, 1)]

                vt = None
                for r0, npairs in plan:
                    b, j0 = divmod(r0, vblock)
                    if j0 == 0:
                        # input rows for this vblock + one chunk of lookahead
                        load_until(min(2 * vblock * (b + 1) + 2 + chunk_rows, h))
                        vt = emit_vblock(b)

                    # 4 PSUM chains (m0..m3), each 3 kw taps, N=512 (2 r's)
                    pts = [
                        ppool.tile([M, npairs, w], mybir.dt.float32,
                                   name="pt", tag="pt", padded_shape=[M, 2, w])
                        for _ in range(4)
                    ]
                    # a=0 last so the final drain->combine chain is short
                    for a in (1, 2, 3, 0):
                        for kw in range(3):
                            nc.tensor.matmul(
                                pts[a][:],
                                wsb[:, a * 3 + kw, :],
                                vt[:, j0 : j0 + npairs, a, kw : kw + w],
                                start=(kw == 0),
                                stop=(kw == 2),
                            )

                    # drain m0..m2 to fp16 (Act), combine (DVE)
                    ad = [
                        spool.tile([M, npairs, w], mybir.dt.float16,
                                   name=f"a{a}", tag=f"a{a}",
                                   padded_shape=[M, 2, w])
                        for a in range(3)
                    ]
                    nc.scalar.copy(ad[1][:], pts[1][:])
                    nc.scalar.copy(ad[2][:], pts[2][:])
                    ot = opool.tile([M, 2 * npairs, w], mybir.dt.float16,
                                    name="ot", tag="ot", padded_shape=[M, 4, w])
                    tt = spool.tile([M, npairs, w], mybir.dt.float16,
                                    name="tt", tag="tt", padded_shape=[M, 2, w])
                    uu = spool.tile([M, npairs, w], mybir.dt.float16,
                                    name="uu", tag="uu", padded_shape=[M, 2, w])
                    nc.vector.tensor_sub(uu[:], ad[1][:], ad[2][:])
                    nc.vector.tensor_sub(ot[:, 1::2, :], uu[:], pts[3][:])
                    nc.scalar.copy(ad[0][:], pts[0][:])
                    nc.vector.tensor_add(tt[:], ad[0][:], ad[1][:])
                    nc.vector.tensor_add(ot[:, 0::2, :], tt[:], ad[2][:])
                    nc.sync.dma_start(
                        out=out_dram[:, 2 * r0 : 2 * r0 + 2 * npairs, :],
                        in_=ot[:],
                    )
                load_until(h)

            for _rep in range(repeats):
                emit_body(first=(_rep == 0))

    nc.compile()
    return nc


def make_weight_tensor(weight, nimg=N_IMG, cin=C_IN, cout=C_OUT):
    """Binarize + F(2,3)-transform + block-diagonalize.

    [cout,cin,3,3] f32 -> [nimg*cin, 12, nimg*cout] fp16 where index
    t = a*3+kw holds u_a[kw]/DIV:
      u0 = w[0], u1 = (w[0]+w[1]+w[2])/2, u2 = (w[0]-w[1]+w[2])/2, u3 = w[2]
    """
    wbin = np.where(weight > 0, 1.0, -1.0).astype(np.float32)  # [co, ci, kh, kw]
    u = np.empty((4, cout, cin, 3), dtype=np.float32)
    u[0] = wbin[:, :, 0, :]
    u[1] = 0.5 * (wbin[:, :, 0, :] + wbin[:, :, 1, :] + wbin[:, :, 2, :])
    u[2] = 0.5 * (wbin[:, :, 0, :] - wbin[:, :, 1, :] + wbin[:, :, 2, :])
    u[3] = wbin[:, :, 2, :]
    u /= DIV
    # [a, co, ci, kw] -> [ci, a*3+kw, co]
    wt = u.transpose(2, 0, 3, 1).reshape(cin, N_WT, cout)
    w12 = np.zeros((nimg * cin, N_WT, nimg * cout), dtype=np.float16)
    for i in range(nimg):
        w12[i * cin : (i + 1) * cin, :, i * cout : (i + 1) * cout] = wt
    return w12


def kernel(x, weight, trace=False, repeats=1, _nc_cache={}):
    """Full-input entry point: x (32,32,256,256) f32, weight (32,32,3,3) f32."""
    x = np.asarray(x, dtype=np.float32)
    x = np.ascontiguousarray(x.astype(np.float16))
    weight = np.asarray(weight, dtype=np.float32)
    n_batch = x.shape[0]
    per_core = n_batch // N_CORES

    if repeats not in _nc_cache:
        _nc_cache[repeats] = build_conv_kernel(repeats=repeats)
    nc = _nc_cache[repeats]

    w12 = make_weight_tensor(weight)
    P = N_IMG * C_IN
    in_maps = [
        {
            "x": x[i * per_core : (i + 1) * per_core].reshape(P, H, W),
            "w12": w12,
        }
        for i in range(N_CORES)
    ]
    try:
        res = bass_utils.run_bass_kernel_spmd(
            nc, in_maps, core_ids=list(range(N_CORES)), trace=trace
        )
    except ModuleNotFoundError:
        res = bass_utils.run_bass_kernel_spmd(
            nc, in_maps, core_ids=list(range(N_CORES)), trace=False
        )
    out = np.concatenate(
        [r["out"].astype(np.float32).reshape(per_core, C_OUT, H, W)
         for r in res.results],
        axis=0,
    )
    if trace:
        kernel.last_results = res
    return out


# revision 39
# speedup vs baseline: 1.4655x; 1.0204x over previous
"""Binarized 3x3 conv (BConv2d) on 8 TRN2 NeuronCores — Winograd F(2,3) along H.

Problem: x (32, 32, 256, 256) f32, weight (32, 32, 3, 3) f32.
  out = conv2d(x, sign(weight), padding='same') / sqrt(32*9)

Strategy:
  - Data-parallel over batch: core i gets images 4i..4i+3 (no collectives).
  - Per core, pack 4 images x 32 input channels onto the 128 SBUF
    partitions; weights are block-diagonal (per-image) as in the direct
    kernel.
  - 1D Winograd F(2,3) over the kh (row) axis: for each output row pair
    (2r, 2r+1), four transformed input rows V0..V3 (cheap row add/subs on
    DVE/Pool) feed 4 PSUM accumulation chains m0..m3, each a 3-tap kw
    convolution (matmuls over K=128).  out[2r] = m0+m1+m2,
    out[2r+1] = m1-m2-m3 (signs/scale folded into the transformed
    weights).  This cuts PE column count 1.5x vs the 9-tap direct form:
    12 matmuls of N=512 per 2 row-pairs instead of 9 of N=512 per pair.
  - Engine split so nothing passes the PE: DVE computes V0/V1 + the
    output combine adds; Pool (GpSimd) computes V2/V3; Act drains
    m0,m1,m2 from PSUM to fp16; DVE's last add reads m3 straight from
    PSUM.
  - fp16 inputs/outputs (host-side fp16 marshalling), fp32 PSUM
    accumulate; transformed weights carry the 1/sqrt(288) scale and the
    /2 Winograd factors.
"""

import numpy as np
import ml_dtypes

import concourse.mybir as mybir
import concourse.tile as tile
from concourse import bacc
from concourse import bass_utils

N_CORES = 8
N_IMG = 4          # images per core
C_IN = 32
C_OUT = 32
K = 3
H = 256
W = 256
DIV = float(np.sqrt(C_IN * K * K))
N_WT = 12          # 4 winograd positions x 3 kw taps


def build_conv_kernel(
    nimg=N_IMG,
    cin=C_IN,
    cout=C_OUT,
    h=H,
    w=W,
    chunk_rows=16,  # input DMA granularity (rows)
    vblock=4,       # r-pairs per V-transform block (8 input rows)
    div=DIV,
    repeats=1,      # execute the whole body N times (for delta-timing)
    warmup_mms=16,  # zero-weight matmuls (N=256) to cover the input wait
):
    """Build the per-core Bass graph.  Returns nc (compiled Bacc)."""
    P = nimg * cin
    assert P <= 128
    M = nimg * cout
    assert M <= 128
    hp, wp = h + 2, w + 2
    nr = h // 2                     # number of output row pairs
    assert nr % vblock == 0

    nc = bacc.Bacc(
        "TRN2", target_bir_lowering=False, debug=False, num_devices=N_CORES
    )
    x_dram = nc.dram_tensor("x", [P, h, w], mybir.dt.float16, kind="ExternalInput")
    w_dram = nc.dram_tensor(
        "w12", [P, N_WT, M], mybir.dt.float16, kind="ExternalInput"
    )
    out_dram = nc.dram_tensor(
        "out", [M, h, w], mybir.dt.float16, kind="ExternalOutput"
    )
    # V tile row pitch: w data cols + 2 pad cols, padded to a multiple of
    # 16 elems (32B SBUF lines) so no V row shares a line with its
    # neighbor.  Rows of every other tile are 512B (w fp16) and therefore
    # line-aligned already.  Misaligned rows let two engines write the
    # same SBUF line concurrently, which corrupts data on real HW.
    wv = w + 16

    with tile.TileContext(nc) as tc:
        with (
            tc.tile_pool(name="persist", bufs=1) as perpool,
            tc.tile_pool(name="stage", bufs=2) as spool,
            tc.tile_pool(name="ostage", bufs=3) as opool,
            tc.tile_pool(name="psum", bufs=8, space="PSUM") as ppool,
        ):
            # x rows with 1-row top/bottom zero padding; no column padding
            # (every row is a single 512B DMA write)
            xpad = perpool.tile([P, hp, w], mybir.dt.float16, name="xpad")
            wsb = perpool.tile([P, N_WT, M], mybir.dt.float16, name="wsb")
            wz = perpool.tile([P, M], mybir.dt.float16, name="wz")
            wmr = perpool.tile([P, w], mybir.dt.float16, name="wmr")
            # manual triple-buffer of V tiles: two blocks of lookahead so
            # the slower Pool V-ops never gate the PE (persistent so the
            # pad-column memsets below cover every buffer once, up front)
            vts = [
                perpool.tile([P, vblock, 4, wv], mybir.dt.float16, name=f"vt{i}")
                for i in range(3)
            ]
            # warmup operands on DVE so the PE can start immediately
            nc.vector.memset(wz[:], 0.0)
            nc.vector.memset(wmr[:], 0.0)

            def emit_input_rows(r0, nrows):
                nc.sync.dma_start(
                    out=xpad[:, r0 + 1 : r0 + nrows + 1, :],
                    in_=x_dram[:, r0 : r0 + nrows, :],
                )

            # first input piece (just enough for the first 2-pair V block),
            # then weights in first-use order (chains run a=1,2,3,0 so taps
            # 3..11 are needed before 0..2)
            first_piece = 6
            emit_input_rows(0, first_piece)
            nc.sync.dma_start(out=wsb[:, 3:, :], in_=w_dram[:, 3:, :])
            nc.sync.dma_start(out=wsb[:, 0:3, :], in_=w_dram[:, 0:3, :])
            # top/bottom zero rows (full 512B rows, DVE: same engine as the
            # first V-block ops that read them)
            nc.vector.memset(xpad[:, 0, :], 0.0)
            nc.vector.memset(xpad[:, hp - 1, :], 0.0)
            # V pad columns (cols 0 and w+1 of each (r, a) row) are zero
            # forever; set them once on Pool while nothing else runs
            for vt_ in vts:
                nc.gpsimd.memset(vt_[:, :, :, 0], 0.0)
                nc.gpsimd.memset(vt_[:, :, :, w + 1], 0.0)

            def emit_body(first):
                next_row = [first_piece if first else 0]

                def load_until(row_needed):
                    while next_row[0] < min(row_needed, h):
                        r0 = next_row[0]
                        # small second piece so the second V block is early
                        nrows = 4 if r0 < 10 else chunk_rows
                        nrows = min(nrows, h - r0)
                        emit_input_rows(r0, nrows)
                        next_row[0] += nrows

                # PE warm-up while the first input chunks are in flight.
                if warmup_mms:
                    wpt = ppool.tile(
                        [M, 2, w], mybir.dt.float32, name="wpt", tag="pt",
                    )
                    for _ in range(warmup_mms):
                        nc.tensor.matmul(
                            wpt[:, 0, 0:w], wz[:], wmr[:],
                            start=True, stop=True,
                        )

                def emit_vblock(bi, p0, n):
                    """V transform for r-pairs [p0, p0+n) into buffer bi%2.

                    V0 = xpad[2r]   - xpad[2r+2]          (DVE)
                    V1 = xpad[2r+1] + xpad[2r+2]          (DVE)
                    V2 = xpad[2r+2] - xpad[2r+1]          (Pool)
                    V3 = xpad[2r+1] - xpad[2r+3]          (Pool)
                    (writes cols 1..w of each V row; cols 0/w+1 are pad)
                    """
                    vt = vts[bi % 3]
                    s = 2 * p0  # xpad row of first d0

                    def rows(off):
                        stop = min(s + off + 2 * n, hp)
                        return xpad[:, s + off : stop : 2, :]

                    # engine assignment: first blocks all-DVE for fast start
                    # (Pool's slow per-op rate would gate the early groups)
                    nc.vector.tensor_sub(vt[:, 0:n, 0, 1 : w + 1], rows(0), rows(2))
                    nc.vector.tensor_add(vt[:, 0:n, 1, 1 : w + 1], rows(1), rows(2))
                    if bi <= 1:
                        nc.vector.tensor_sub(vt[:, 0:n, 2, 1 : w + 1],
                                             rows(2), rows(1))
                        nc.vector.tensor_sub(vt[:, 0:n, 3, 1 : w + 1],
                                             rows(1), rows(3))
                    else:
                        # Pool is slow per-op: emit in 2-pair halves so the
                        # first consumer group isn't gated on the whole block
                        for q0 in range(0, n, 2):
                            q1 = min(q0 + 2, n)
                            sq = s + 2 * q0

                            def qrows(off):
                                stop = min(sq + off + 2 * (q1 - q0), hp)
                                return xpad[:, sq + off : stop : 2, :]

                            nc.gpsimd.tensor_sub(
                                vt[:, q0:q1, 2, 1 : w + 1], qrows(2), qrows(1))
                            nc.gpsimd.tensor_sub(
                                vt[:, q0:q1, 3, 1 : w + 1], qrows(1), qrows(3))
                    return vt

                # groups of 2 r-pairs; final group split in two 1-pair
                # groups so the post-matmul drain tail is short
                plan = [(r0, 2) for r0 in range(0, nr - 2, 2)]
                plan += [(nr - 2, 1), (nr - 1, 1)]

                # V blocks: two small first blocks so the PE starts early
                vplan = [(0, 2), (2, 2)]
                p = 4
                while p < nr:
                    vplan.append((p, vblock))
                    p += vblock

                def emit_next_vblock():
                    p0, n = vplan[len(vtiles)]
                    # input rows for this vblock + one chunk of lookahead
                    load_until(min(2 * (p0 + n) + 2 + chunk_rows, h))
                    vtiles.append((p0, emit_vblock(len(vtiles), p0, n)))

                # bootstrap all three buffers; thereafter emit block cur+2
                # at the first group of block cur (all consumers of block
                # cur-1 are then emitted, so the 3-deep buffer WAR is safe)
                vtiles = []   # (p0, vt) per emitted block
                emit_next_vblock()
                emit_next_vblock()
                emit_next_vblock()
                # pre-issue the next input chunks before any output DMAs
                # exist to head-of-line-block them on the DMA sequencer
                load_until(2 * vblock * 2 + 2 + 2 * chunk_rows)
                cur = 0
                for r0, npairs in plan:
                    if cur + 1 < len(vtiles) and vtiles[cur + 1][0] <= r0:
                        cur += 1
                        if len(vtiles) < len(vplan):
                            emit_next_vblock()
                    vbase, vt = vtiles[cur]
                    j0 = r0 - vbase

                    # 4 PSUM chains (m0..m3), each 3 kw taps, N=512 (2 r's)
                    pts = [
                        ppool.tile([M, npairs, w], mybir.dt.float32,
                                   name="pt", tag="pt", padded_shape=[M, 2, w])
                        for _ in range(4)
                    ]
                    # a=0 last so the drain->combine chain overlaps the
                    # a=3 matmuls; for the very last group a=3 goes last
                    # instead, so the tail ends on the single short
                    # ot-odd op (uu is ready before the a=3 chain ends)
                    last = r0 == nr - 1
                    for a in ((1, 2, 0, 3) if last else (1, 2, 3, 0)):
                        for kw in range(3):
                            nc.tensor.matmul(
                                pts[a][:],
                                wsb[:, a * 3 + kw, :],
                                vt[:, j0 : j0 + npairs, a, kw : kw + w],
                                start=(kw == 0),
                                stop=(kw == 2),
                            )

                    # drain m0..m2 to fp16 (Act), combine (DVE)
                    ad = [
                        spool.tile([M, npairs, w], mybir.dt.float16,
                                   name=f"a{a}", tag=f"a{a}",
                                   padded_shape=[M, 2, w])
                        for a in range(3)
                    ]
                    nc.scalar.copy(ad[1][:], pts[1][:])
                    nc.scalar.copy(ad[2][:], pts[2][:])
                    ot = opool.tile([M, 2 * npairs, w], mybir.dt.float16,
                                    name="ot", tag="ot", padded_shape=[M, 4, w])
                    tt = spool.tile([M, npairs, w], mybir.dt.float16,
                                    name="tt", tag="tt", padded_shape=[M, 2, w])
                    uu = spool.tile([M, npairs, w], mybir.dt.float16,
                                    name="uu", tag="uu", padded_shape=[M, 2, w])
                    nc.vector.tensor_sub(uu[:], ad[1][:], ad[2][:])
                    if last:
                        nc.scalar.copy(ad[0][:], pts[0][:])
                        nc.vector.tensor_add(tt[:], ad[0][:], ad[1][:])
                        nc.vector.tensor_add(ot[:, 0::2, :], tt[:], ad[2][:])
                        nc.sync.dma_start(
                            out=out_dram[:, 2 * r0 : 2 * r0 + 1, :],
                            in_=ot[:, 0:1, :],
                        )
                        nc.vector.tensor_sub(ot[:, 1::2, :], uu[:], pts[3][:])
                        nc.sync.dma_start(
                            out=out_dram[:, 2 * r0 + 1 : 2 * r0 + 2, :],
                            in_=ot[:, 1:2, :],
                        )
                    else:
                        nc.vector.tensor_sub(ot[:, 1::2, :], uu[:], pts[3][:])
                        nc.scalar.copy(ad[0][:], pts[0][:])
                        nc.vector.tensor_add(tt[:], ad[0][:], ad[1][:])
                        nc.vector.tensor_add(ot[:, 0::2, :], tt[:], ad[2][:])
                        nc.sync.dma_start(
                            out=out_dram[:, 2 * r0 : 2 * r0 + 2 * npairs, :],
                            in_=ot[:],
                        )
                load_until(h)

            for _rep in range(repeats):
                emit_body(first=(_rep == 0))

    nc.compile()
    return nc


def make_weight_tensor(weight, nimg=N_IMG, cin=C_IN, cout=C_OUT):
    """Binarize + F(2,3)-transform + block-diagonalize.

    [cout,cin,3,3] f32 -> [nimg*cin, 12, nimg*cout] fp16 where index
    t = a*3+kw holds u_a[kw]/DIV:
      u0 = w[0], u1 = (w[0]+w[1]+w[2])/2, u2 = (w[0]-w[1]+w[2])/2, u3 = w[2]
    """
    wbin = np.where(weight > 0, 1.0, -1.0).astype(np.float32)  # [co, ci, kh, kw]
    u = np.empty((4, cout, cin, 3), dtype=np.float32)
    u[0] = wbin[:, :, 0, :]
    u[1] = 0.5 * (wbin[:, :, 0, :] + wbin[:, :, 1, :] + wbin[:, :, 2, :])
    u[2] = 0.5 * (wbin[:, :, 0, :] - wbin[:, :, 1, :] + wbin[:, :, 2, :])
    u[3] = wbin[:, :, 2, :]
    u /= DIV
    # [a, co, ci, kw] -> [ci, a*3+kw, co]
    wt = u.transpose(2, 0, 3, 1).reshape(cin, N_WT, cout)
    w12 = np.zeros((nimg * cin, N_WT, nimg * cout), dtype=np.float16)
    for i in range(nimg):
        w12[i * cin : (i + 1) * cin, :, i * cout : (i + 1) * cout] = wt
    return w12


def kernel(x, weight, trace=False, repeats=1, _nc_cache={}):
    """Full-input entry point: x (32,32,256,256) f32, weight (32,32,3,3) f32."""
    x = np.asarray(x, dtype=np.float32)
    x = np.ascontiguousarray(x.astype(np.float16))
    weight = np.asarray(weight, dtype=np.float32)
    n_batch = x.shape[0]
    per_core = n_batch // N_CORES

    if repeats not in _nc_cache:
        _nc_cache[repeats] = build_conv_kernel(repeats=repeats)
    nc = _nc_cache[repeats]

    w12 = make_weight_tensor(weight)
    P = N_IMG * C_IN
    in_maps = [
        {
            "x": x[i * per_core : (i + 1) * per_core].reshape(P, H, W),
            "w12": w12,
        }
        for i in range(N_CORES)
    ]
    try:
        res = bass_utils.run_bass_kernel_spmd(
            nc, in_maps, core_ids=list(range(N_CORES)), trace=trace
        )
    except ModuleNotFoundError:
        res = bass_utils.run_bass_kernel_spmd(
            nc, in_maps, core_ids=list(range(N_CORES)), trace=False
        )
    out = np.concatenate(
        [r["out"].astype(np.float32).reshape(per_core, C_OUT, H, W)
         for r in res.results],
        axis=0,
    )
    if trace:
        kernel.last_results = res
    return out
